# revision 1
# baseline (speedup 1.0000x reference)
"""ChebConv (K=3, two layers + softmax) GNN kernel for 8 Trainium2 NeuronCores.

Strategy (dst-node sharding, graph replicated):
  - Nodes are split into 8 contiguous shards (one per core); each core owns the
    edges whose *destination* lies in its shard.
  - Host preprocessing sorts edges by dst, groups them per 128-dst block, and
    pads each block's edge list to a whole number of 128-edge chunks.  Chunk
    counts per block are maxed across cores so a single SPMD program serves
    all 8 cores (padding edges have w=0 and gather table row 0).
  - SpMM per chunk: a 128-row indirect DMA gathers x[src[e]] (256B rows) from
    a replicated table in DRAM; the DVE builds S_T[e, d] = (iota==dst_local)*w
    in one dual-op tensor_scalar; the PE accumulates X_g.T @ S_T into PSUM,
    yielding feature-major [64, 128] output blocks that feed the dense W_k
    matmuls directly (contraction over features needs features on partitions).
  - T2 = 2*A*T1 - T0 stays on chip (the factor 2 is folded into a second edge
    weight array); T1-type outputs are PE-transposed back to row-major and
    stored as the next gather table.  Feature-major copies of x/T1/h/T3 blocks
    bounce through DRAM between phases (bulk DMA, cheap next to the gathers).
  - Tables computed on device (T1, h, T3) are exchanged with AllGather
    collectives (3.2MB per core each) between SpMM phases.
  - Bias adds are folded into ACT activations (Relu for layer 1, Identity for
    layer 2); softmax runs per 128-node block after a final PE transpose.
"""

import os

import numpy as np

import concourse.bass as bass
import concourse.mybir as mybir
import concourse.tile as tile
from concourse import bacc
from concourse.bass import IndirectOffsetOnAxis
from concourse.bass_utils import run_bass_kernel_spmd

NCORES = 8
P = 128
NQ = 4  # SWDGE queues used round-robin for the gather stream

F32 = mybir.dt.float32
I32 = mybir.dt.int32
ALU = mybir.AluOpType
ACTF = mybir.ActivationFunctionType


def _preprocess(x, edge_index):
    N, F = x.shape
    assert N % NCORES == 0
    PN = N // NCORES
    NB = (PN + P - 1) // P
    PNP = NB * P
    TROWS = NCORES * PNP

    src = edge_index[0].astype(np.int64)
    dst = edge_index[1].astype(np.int64)
    keep = src != dst
    deg = np.bincount(src[keep], minlength=N).astype(np.float32)
    dis = np.where(deg > 0, 1.0 / np.sqrt(np.maximum(deg, 1.0)), 0.0).astype(
        np.float32
    )
    w = np.where(keep, -dis[src] * dis[dst], 0.0).astype(np.float32)

    order = np.argsort(dst, kind="stable")
    s_s, s_d, s_w = src[order], dst[order], w[order]
    core_of = s_d // PN
    dl = s_d % PN
    blk = dl // P
    dstloc = (dl % P).astype(np.float32)

    # gather-table row of each edge's source node, and its int16 sub-table
    trow = ((s_s // PN) * PNP + (s_s % PN)).astype(np.int64)
    QROWS = TROWS // 4
    assert QROWS <= 32768
    quad = trow // QROWS

    # bucket edges by (core, block, src-quadrant); order within bucket free
    cnt = np.zeros((NCORES, NB, 4), np.int64)
    np.add.at(cnt, (core_of, blk, quad), 1)
    cbq = -(-cnt.max(axis=0) // P)            # [NB, 4] chunks per bucket
    cbq[:, 0] = np.maximum(cbq[:, 0], 1)      # each block needs >= 1 chunk
    cb = cbq.sum(axis=1)                      # chunks per block
    CT = int(cb.sum())
    cbase_q = np.zeros((NB, 4), np.int64)
    flat = cbq.ravel()
    off = np.zeros(NB * 4, np.int64)
    off[1:] = np.cumsum(flat)[:-1]
    cbase_q = off.reshape(NB, 4)

    key = (core_of * NB + blk) * 4 + quad
    korder = np.argsort(key, kind="stable")
    starts = np.zeros(NCORES * NB * 4 + 1, np.int64)
    starts[1:] = np.cumsum(cnt.ravel())
    starts = starts[:-1].reshape(NCORES, NB, 4)

    t_r, d_l, w_s = trow[korder], dstloc[korder], s_w[korder]
    eidx16 = np.zeros((NCORES, P, CT * 8), np.int16)
    edst = np.zeros((NCORES, P, CT), np.float32)
    ew = np.zeros((NCORES, P, CT), np.float32)
    base_ptr = 0
    for c in range(NCORES):
        for b in range(NB):
            for q in range(4):
                n = int(cnt[c, b, q])
                s0 = int(starts[c, b, q])
                if n:
                    js = np.arange(n)
                    cols = cbase_q[b, q] + js // P
                    parts = js % P
                    edst[c, parts, cols] = d_l[s0 : s0 + n]
                    ew[c, parts, cols] = w_s[s0 : s0 + n]
                    # int16 local rows, wrapped 16-wide, padded to chunk size
                    npad = int(cbq[b, q]) * P
                    loc = np.zeros(npad, np.int64)
                    loc[:n] = t_r[s0 : s0 + n] - q * QROWS
                    wrap = loc.reshape(-1, 16).T.astype(np.int16)  # [16, npad/16]
                    c0 = cbase_q[b, q] * 8
                    eidx16[c, :, c0 : c0 + npad // 16] = np.tile(wrap, (8, 1))
                elif int(cbq[b, q]):
                    pass  # all-padding chunk: zeros already there

    # replicated padded gather table for x; per-core feature-major x blocks
    x_rm = np.zeros((TROWS, F), np.float32)
    xfm = np.zeros((NCORES, NB, F, P), np.float32)
    for c in range(NCORES):
        x_rm[c * PNP : c * PNP + PN] = x[c * PN : (c + 1) * PN]
        xpad = np.zeros((PNP, F), np.float32)
        xpad[:PN] = x[c * PN : (c + 1) * PN]
        xfm[c] = xpad.reshape(NB, P, F).transpose(0, 2, 1)

    return dict(
        PN=PN, NB=NB, PNP=PNP, TROWS=TROWS, F=F, cb=cb.tolist(),
        cbq=cbq.tolist(), cbase_q=cbase_q.tolist(), CT=CT,
        eidx16=eidx16, edst=edst, edstn=-edst, ew=ew, x_rm=x_rm, xfm=xfm,
    )


def _build(meta, K, HID, NCLS):
    NB, PNP, TROWS, F, CT = (
        meta["NB"], meta["PNP"], meta["TROWS"], meta["F"], meta["CT"]
    )
    cb = meta["cb"]
    cbq, cbase_q = meta["cbq"], meta["cbase_q"]
    QROWS = TROWS // 4

    nc = bacc.Bacc(
        "TRN2", target_bir_lowering=False, debug=False,
        num_devices=NCORES, num_swdge_queues=NQ,
    )
    d_xrm = nc.dram_tensor("x_rm", [TROWS, F], F32, kind="ExternalInput")
    d_xfm = nc.dram_tensor("xfm", [NB, F, P], F32, kind="ExternalInput")
    d_W1 = nc.dram_tensor("W1", [K, F, HID], F32, kind="ExternalInput")
    d_W2 = nc.dram_tensor("W2", [K, HID, NCLS], F32, kind="ExternalInput")
    d_b1 = nc.dram_tensor("b1", [HID, 1], F32, kind="ExternalInput")
    d_b2 = nc.dram_tensor("b2", [NCLS, 1], F32, kind="ExternalInput")
    d_iota = nc.dram_tensor("iota", [P, P], F32, kind="ExternalInput")
    d_ident = nc.dram_tensor("ident", [P, P], F32, kind="ExternalInput")
    d_eidx = nc.dram_tensor("eidx", [P, CT * 8], mybir.dt.int16, kind="ExternalInput")
    d_edst = nc.dram_tensor("edst", [P, CT], F32, kind="ExternalInput")
    d_edstn = nc.dram_tensor("edstn", [P, CT], F32, kind="ExternalInput")
    d_ew = nc.dram_tensor("ew", [P, CT], F32, kind="ExternalInput")
    d_out = nc.dram_tensor("out", [PNP, NCLS], F32, kind="ExternalOutput")

    d_t1loc = nc.dram_tensor("t1loc", [PNP, F], F32)
    d_hloc = nc.dram_tensor("hloc", [PNP, F], F32)
    d_t3loc = nc.dram_tensor("t3loc", [PNP, F], F32)
    d_t1fm = nc.dram_tensor("t1fm", [NB, F, P], F32)
    d_hfm = nc.dram_tensor("hfm", [NB, F, P], F32)
    d_t3fm = nc.dram_tensor("t3fm", [NB, F, P], F32)
    d_t1full = nc.dram_tensor("t1full", [TROWS, F], F32, addr_space="Shared")
    d_hfull = nc.dram_tensor("hfull", [TROWS, F], F32, addr_space="Shared")
    d_t3full = nc.dram_tensor("t3full", [TROWS, F], F32, addr_space="Shared")

    groups = [list(range(NCORES))]
    qcounter = [0]

    with tile.TileContext(nc) as tc:
        with (
            tc.tile_pool(name="const", bufs=1) as constp,
            tc.tile_pool(name="big", bufs=1) as bigp,
            tc.tile_pool(name="xg", bufs=12) as xgp,
            tc.tile_pool(name="st", bufs=6) as stp,
            tc.tile_pool(name="fm", bufs=3) as fmp,
            tc.tile_pool(name="tmp", bufs=4) as tmpp,
            tc.tile_pool(name="sm", bufs=4) as smp,
            tc.tile_pool(name="pseg", bufs=2, space="PSUM") as psegp,
            tc.tile_pool(name="pden", bufs=2, space="PSUM") as pdenp,
            tc.tile_pool(name="ptr", bufs=2, space="PSUM") as ptrp,
        ):
            # ---- constants / resident tiles ----
            iota = constp.tile([P, P], F32, tag="iota")
            ident = constp.tile([P, P], F32, tag="ident")
            w1k = [constp.tile([F, HID], F32, tag=f"w1_{k}", name=f"w1_{k}")
                   for k in range(K)]
            w2k = [constp.tile([HID, NCLS], F32, tag=f"w2_{k}", name=f"w2_{k}")
                   for k in range(K)]
            b1c = constp.tile([HID, 1], F32, tag="b1c")
            b2c = constp.tile([NCLS, 1], F32, tag="b2c")
            eidx = bigp.tile([P, CT * 8], mybir.dt.int16, tag="eidx")
            edst = bigp.tile([P, CT], F32, tag="edst")
            edstn = bigp.tile([P, CT], F32, tag="edstn")
            ew1 = bigp.tile([P, CT], F32, tag="ew1")
            ew2 = bigp.tile([P, CT], F32, tag="ew2")

            nc.sync.dma_start(iota[:], d_iota[:])
            nc.sync.dma_start(ident[:], d_ident[:])
            for k in range(K):
                nc.sync.dma_start(w1k[k][:], d_W1[k])
                nc.sync.dma_start(w2k[k][:], d_W2[k])
            nc.sync.dma_start(b1c[:], d_b1[:])
            nc.sync.dma_start(b2c[:], d_b2[:])
            nc.sync.dma_start(eidx[:], d_eidx[:])
            nc.sync.dma_start(edst[:], d_edst[:])
            nc.sync.dma_start(edstn[:], d_edstn[:])
            nc.sync.dma_start(ew1[:], d_ew[:])
            nc.vector.tensor_scalar(
                out=ew2[:], in0=ew1[:], scalar1=2.0, scalar2=None, op0=ALU.mult
            )

            def seg_block(b, table, wcols):
                """SpMM for dst-block b via dma_gather over the four int16
                sub-tables; returns PSUM tile [F, 128]."""
                seg = psegp.tile([F, P], F32, tag="seg", name=f"seg{b}")
                total = cb[b]
                done = 0
                for q in range(4):
                    kq = cbq[b][q]
                    cq = 0
                    while cq < kq:
                        k = min(8, kq - cq)
                        c0 = cbase_q[b][q] + cq
                        xg = xgp.tile([P, 8, F], F32, tag="xg",
                                      name=f"xg{b}_{q}_{cq}")
                        nc.gpsimd.dma_gather(
                            out_ap=xg[:, :k, :],
                            in_ap=table[q * QROWS : (q + 1) * QROWS, :],
                            idxs_ap=eidx[:, c0 * 8 : (c0 + k) * 8],
                            num_idxs=k * P, num_idxs_reg=k * P,
                            elem_size=F,
                            queue_num=qcounter[0] % NQ,
                        )
                        qcounter[0] += 1
                        for j in range(k):
                            c = c0 + j
                            st = stp.tile([P, P], F32, tag="st",
                                          name=f"st{b}_{q}_{cq}_{j}")
                            if qcounter[0] % 4 == 0:
                                # DVE path: one-hot carries the edge weight
                                nc.vector.tensor_scalar(
                                    out=st[:], in0=iota[:],
                                    scalar1=edst[:, c : c + 1],
                                    scalar2=wcols[:, c : c + 1],
                                    op0=ALU.is_equal, op1=ALU.mult,
                                )
                                lhs = xg[:, j, :]
                            else:
                                # ACT path: pure 0/1 one-hot via
                                # relu(1 - (iota - dst)^2); w applied to the
                                # gathered rows on DVE ([128,64], half the
                                # elements of the one-hot build).
                                sq = stp.tile([P, P], F32, tag="sq",
                                              name=f"sq{b}_{q}_{cq}_{j}")
                                nc.scalar.activation(
                                    sq[:], iota[:], ACTF.Square,
                                    bias=edstn[:, c : c + 1], scale=1.0,
                                )
                                nc.scalar.activation(
                                    st[:], sq[:], ACTF.Relu,
                                    bias=1.0, scale=-1.0,
                                )
                                xgw = stp.tile([P, F], F32, tag="xgw",
                                               name=f"xgw{b}_{q}_{cq}_{j}")
                                nc.vector.tensor_scalar(
                                    out=xgw[:], in0=xg[:, j, :],
                                    scalar1=wcols[:, c : c + 1], scalar2=None,
                                    op0=ALU.mult,
                                )
                                lhs = xgw[:]
                            nc.tensor.matmul(
                                seg[:], lhsT=lhs, rhs=st[:],
                                start=(done == 0), stop=(done == total - 1),
                            )
                            done += 1
                        cq += k
                return seg

            def store_rowmajor(b, fm_tile, dest):
                """Transpose a feature-major [F,128] SBUF tile and store it
                row-major [128,F] into DRAM table `dest`."""
                tr = ptrp.tile([P, F], F32, tag="tr", name=f"tr{b}")
                nc.tensor.transpose(tr[:], fm_tile, ident[:F, :F])
                rm = tmpp.tile([P, F], F32, tag="rm", name=f"rm{b}")
                nc.scalar.copy(rm[:], tr[:])
                nc.sync.dma_start(dest[b * P : (b + 1) * P, :], rm[:])

            # ---- phase 1: T1 = A @ x ----
            for b in range(NB):
                seg = seg_block(b, d_xrm, ew1)
                t1b = fmp.tile([F, P], F32, tag="fmt", name=f"t1b{b}")
                nc.vector.tensor_copy(t1b[:], seg[:])
                nc.sync.dma_start(d_t1fm[b], t1b[:])
                store_rowmajor(b, t1b[:], d_t1loc)
            nc.gpsimd.collective_compute(
                "AllGather", ALU.bypass, replica_groups=groups,
                ins=[d_t1loc[:]], outs=[d_t1full[:]],
            )

            # ---- phase 2: T2 = 2 A T1 - x;  h = relu(sum_k Tk @ W1k + b1) ----
            for b in range(NB):
                seg = seg_block(b, d_t1full, ew2)
                xfmb = fmp.tile([F, P], F32, tag="fmt", name=f"xfmb{b}")
                nc.sync.dma_start(xfmb[:], d_xfm[b])
                t1fmb = fmp.tile([F, P], F32, tag="fmt", name=f"t1fmb{b}")
                nc.sync.dma_start(t1fmb[:], d_t1fm[b])
                tx2 = tmpp.tile([F, P], F32, tag="tx2", name=f"tx2_{b}")
                nc.vector.tensor_tensor(
                    out=tx2[:], in0=seg[:], in1=xfmb[:], op=ALU.subtract
                )
                o1 = pdenp.tile([HID, P], F32, tag="oden", name=f"o1_{b}")
                nc.tensor.matmul(o1[:], lhsT=w1k[0][:], rhs=xfmb[:],
                                 start=True, stop=False)
                nc.tensor.matmul(o1[:], lhsT=w1k[1][:], rhs=t1fmb[:],
                                 start=False, stop=False)
                nc.tensor.matmul(o1[:], lhsT=w1k[2][:], rhs=tx2[:],
                                 start=False, stop=True)
                hb = fmp.tile([F, P], F32, tag="fmt", name=f"hb{b}")
                nc.scalar.activation(hb[:], o1[:], ACTF.Relu, bias=b1c[:])
                nc.sync.dma_start(d_hfm[b], hb[:])
                store_rowmajor(b, hb[:], d_hloc)
            nc.gpsimd.collective_compute(
                "AllGather", ALU.bypass, replica_groups=groups,
                ins=[d_hloc[:]], outs=[d_hfull[:]],
            )

            # ---- phase 3: T3 = A @ h ----
            for b in range(NB):
                seg = seg_block(b, d_hfull, ew1)
                t3b = fmp.tile([F, P], F32, tag="fmt", name=f"t3b{b}")
                nc.vector.tensor_copy(t3b[:], seg[:])
                nc.sync.dma_start(d_t3fm[b], t3b[:])
                store_rowmajor(b, t3b[:], d_t3loc)
            nc.gpsimd.collective_compute(
                "AllGather", ALU.bypass, replica_groups=groups,
                ins=[d_t3loc[:]], outs=[d_t3full[:]],
            )

            # ---- phase 4: out = softmax(sum_k Tk @ W2k + b2) ----
            for b in range(NB):
                seg = seg_block(b, d_t3full, ew2)
                hfmb = fmp.tile([F, P], F32, tag="fmt", name=f"hfmb{b}")
                nc.sync.dma_start(hfmb[:], d_hfm[b])
                t3fmb = fmp.tile([F, P], F32, tag="fmt", name=f"t3fmb{b}")
                nc.sync.dma_start(t3fmb[:], d_t3fm[b])
                th2 = tmpp.tile([F, P], F32, tag="tx2", name=f"th2_{b}")
                nc.vector.tensor_tensor(
                    out=th2[:], in0=seg[:], in1=hfmb[:], op=ALU.subtract
                )
                o2 = pdenp.tile([NCLS, P], F32, tag="oden", name=f"o2_{b}")
                nc.tensor.matmul(o2[:], lhsT=w2k[0][:], rhs=hfmb[:],
                                 start=True, stop=False)
                nc.tensor.matmul(o2[:], lhsT=w2k[1][:], rhs=t3fmb[:],
                                 start=False, stop=False)
                nc.tensor.matmul(o2[:], lhsT=w2k[2][:], rhs=th2[:],
                                 start=False, stop=True)
                o2b = tmpp.tile([NCLS, P], F32, tag="o2b", name=f"o2b{b}")
                nc.scalar.activation(o2b[:], o2[:], ACTF.Identity, bias=b2c[:])
                tr2 = ptrp.tile([P, NCLS], F32, tag="tr", name=f"tr2_{b}")
                nc.tensor.transpose(tr2[:], o2b[:], ident[:NCLS, :NCLS])
                o2t = smp.tile([P, NCLS], F32, tag="o2t", name=f"o2t{b}")
                nc.vector.tensor_copy(o2t[:], tr2[:])
                negm = smp.tile([P, 1], F32, tag="negm", name=f"negm{b}")
                nc.vector.tensor_reduce(
                    negm[:], o2t[:], axis=mybir.AxisListType.X,
                    op=ALU.max, negate=True,
                )
                ex = smp.tile([P, NCLS], F32, tag="ex", name=f"ex{b}")
                nc.scalar.activation(ex[:], o2t[:], ACTF.Exp, bias=negm[:])
                ssum = smp.tile([P, 1], F32, tag="ssum", name=f"ssum{b}")
                nc.vector.tensor_reduce(
                    ssum[:], ex[:], axis=mybir.AxisListType.X, op=ALU.add
                )
                rcp = smp.tile([P, 1], F32, tag="rcp", name=f"rcp{b}")
                nc.vector.reciprocal(rcp[:], ssum[:])
                res = smp.tile([P, NCLS], F32, tag="res", name=f"res{b}")
                nc.vector.tensor_scalar(
                    out=res[:], in0=ex[:], scalar1=rcp[:, :1], scalar2=None,
                    op0=ALU.mult,
                )
                nc.sync.dma_start(d_out[b * P : (b + 1) * P, :], res[:])

    nc.compile()
    return nc


def kernel(x, edge_index, W1, b1, W2, b2, _backend="hw"):
    x = np.asarray(x, dtype=np.float32)
    edge_index = np.asarray(edge_index, dtype=np.int32)
    W1 = np.asarray(W1, dtype=np.float32)
    b1 = np.asarray(b1, dtype=np.float32)
    W2 = np.asarray(W2, dtype=np.float32)
    b2 = np.asarray(b2, dtype=np.float32)
    K, F, HID = W1.shape
    NCLS = W2.shape[2]

    meta = _preprocess(x, edge_index)
    nc = _build(meta, K, HID, NCLS)

    iota = np.tile(np.arange(P, dtype=np.float32), (P, 1))
    ident = np.eye(P, dtype=np.float32)
    in_maps = []
    for c in range(NCORES):
        in_maps.append({
            "x_rm": meta["x_rm"], "xfm": meta["xfm"][c],
            "W1": W1, "W2": W2,
            "b1": b1.reshape(-1, 1).astype(np.float32),
            "b2": b2.reshape(-1, 1).astype(np.float32),
            "iota": iota, "ident": ident,
            "eidx": meta["eidx16"][c], "edst": meta["edst"][c], "edstn": meta["edstn"][c],
            "ew": meta["ew"][c],
        })

    PN = meta["PN"]
    if _backend == "sim":
        from concourse.bass_interp import MultiCoreSim

        sim = MultiCoreSim(nc, num_cores=NCORES)
        for c in range(NCORES):
            for name, arr in in_maps[c].items():
                sim.cores[c].tensor(name)[:] = arr
        sim.simulate()
        outs = [np.array(sim.cores[c].tensor("out"))[:PN] for c in range(NCORES)]
        kernel.last_result = None
        return np.concatenate(outs, axis=0)

    trace = bool(os.environ.get("BASS_TRACE"))
    res = run_bass_kernel_spmd(
        nc, in_maps, core_ids=list(range(NCORES)), trace=trace
    )
    kernel.last_result = res
    return np.concatenate(
        [res.results[c]["out"][:PN] for c in range(NCORES)], axis=0
    )



# revision 7
# speedup vs baseline: 1.5234x; 1.5234x over previous
"""ChebConv (K=3, two layers + softmax) GNN kernel for 8 Trainium2 NeuronCores.

Strategy (dst-node sharding, ELL-format SpMM, quad-packed bf16 tables):
  - Nodes are split into 8 contiguous shards (one per core); each core owns the
    edges whose *destination* lies in its shard.
  - Within each core, nodes are PERMUTED by descending in-degree so that each
    128-node dst block has near-uniform degree; edges are laid out in ELL
    format: slot grid [128 dst-partitions x L_b chunks] per block, padded with
    (idx=0, w=0).  Chunk counts L_b are maxed across cores so one SPMD program
    serves all 8.
  - Gather tables are QUAD-PACKED bf16: [TROWS/4, 256] -- 4 nodes per 512B row
    so the row index fits dma_gather's int16 (25088 < 32768).  One 512B
    descriptor per edge slot; the needed sub-node is selected by 4 masked
    per-edge weight arrays wk[k] (w where trow%4==k else 0).
  - SpMM per ~GMAX-chunk group: one dma_gather + 4 DVE broadcast-multiplies
    (one per sub-node lane) into a [128, 4, GMAX, 64] product buffer, then ONE
    strided 4D DVE reduce per block over (lane, slot) -> row-major [128, F].
    No per-chunk one-hot builds, no PE matmuls in the SpMM.
  - Chebyshev algebra is refactored so every phase gathers a fresh table with
    the SAME edge weights (the 2x of T2 = 2 A T1 - T0 is folded into W):
      h   = relu(x(W1_0-W1_2) + T1 W1_1 + A v + b1),   v = T1 (2 W1_2)
      out = softmax(h(W2_0-W2_2) + S1 W2_1 + A u + b2), u = S1 (2 W2_2)
    Phase tables: x, v, h (64-wide), u (40-wide, padded to 64 in the quad).
  - Dense per-block work: PE transpose + 3 small bf16 matmuls (biases folded
    in via a ones-row matmul).  Feature-major copies bounce through DRAM.
  - v/h/u tables are exchanged with AllGather collectives split into 2 slices
    each so the first slice overlaps the tail of the producing phase.
"""

import os

import numpy as np
import ml_dtypes

import concourse.bass as bass
import concourse.mybir as mybir
import concourse.tile as tile
from concourse import bacc
from concourse.bass_utils import run_bass_kernel_spmd

NCORES = 8
P = 128
GMAX = 48  # ELL chunks per gather group (one product buffer)
CPG = 8    # chunks per dma_gather call (1024 idxs -- SWDGE ring limit)
NQ = 4     # SWDGE queues round-robin

BF16 = mybir.dt.bfloat16
F32 = mybir.dt.float32
I16 = mybir.dt.int16
ALU = mybir.AluOpType
ACTF = mybir.ActivationFunctionType
AX = mybir.AxisListType
NPBF = ml_dtypes.bfloat16


def _preprocess(x, edge_index):
    N, F = x.shape
    assert N % NCORES == 0
    PN = N // NCORES
    NB = (PN + P - 1) // P
    PNP = NB * P
    TROWS = NCORES * PNP
    assert TROWS % 4 == 0 and TROWS // 4 < 32768

    src = edge_index[0].astype(np.int64)
    dst = edge_index[1].astype(np.int64)
    keep = src != dst
    deg = np.bincount(src[keep], minlength=N).astype(np.float64)
    dis = np.where(deg > 0, 1.0 / np.sqrt(np.maximum(deg, 1.0)), 0.0)
    w = np.where(keep, -dis[src] * dis[dst], 0.0).astype(np.float32)
    live = w != 0.0
    src, dst, w = src[live], dst[live], w[live]

    # per-core rank of each node: descending in-degree within its core
    indeg = np.bincount(dst, minlength=N).astype(np.int64)
    rank = np.empty(N, np.int64)
    for c in range(NCORES):
        ids = np.arange(c * PN, (c + 1) * PN)
        order = np.argsort(-indeg[ids], kind="stable")
        rank[ids[order]] = np.arange(PN)

    # AllGather slice structure (2 slices of whole blocks; rows %4 == 0)
    if NB >= 2:
        sl_blocks = [NB // 2, NB - NB // 2]
    else:
        sl_blocks = [NB]
    sl_rows = np.array([b * P for b in sl_blocks], np.int64)
    sl_local0 = np.concatenate([[0], np.cumsum(sl_rows)])[:-1]
    sl_base = np.concatenate([[0], np.cumsum(sl_rows * NCORES)])[:-1]

    c_all = np.arange(N) // PN
    r_all = rank
    s_all = np.searchsorted(np.cumsum(sl_rows), r_all, side="right")
    trow = sl_base[s_all] + c_all * sl_rows[s_all] + (r_all - sl_local0[s_all])

    # ELL slot grid
    blk = (rank[dst] // P).astype(np.int64)
    part = (rank[dst] % P).astype(np.int64)
    order = np.argsort(dst, kind="stable")
    sdst = dst[order]
    starts = np.flatnonzero(np.r_[True, sdst[1:] != sdst[:-1]])
    seg_len = np.diff(np.r_[starts, len(sdst)])
    ccount = np.arange(len(sdst)) - np.repeat(starts, seg_len)
    slot = np.empty(len(dst), np.int64)
    slot[order] = ccount

    indeg_by_rank = np.zeros(NCORES * PNP, np.int64)
    indeg_by_rank[c_all * PNP + r_all] = indeg
    Lb = indeg_by_rank.reshape(NCORES, NB, P).max(axis=(0, 2))
    Lb = np.maximum(Lb, 1)
    cb_off = np.concatenate([[0], np.cumsum(Lb)])
    CT = int(cb_off[-1])

    core_of = dst // PN
    col = cb_off[blk] + slot
    qidx = np.zeros((NCORES, P, CT), np.int64)      # quad row of src
    wk = np.zeros((NCORES, 4, P, CT), np.float32)   # lane-masked weights
    qidx[core_of, part, col] = trow[src] // 4
    wk[core_of, trow[src] % 4, part, col] = w

    # wrapped int16 index stream per core: position i=(c*128+d) -> [i%16,i//16]
    eidx16 = np.zeros((NCORES, P, CT * 8), np.int16)
    for c in range(NCORES):
        stream = qidx[c].T.reshape(-1)              # chunk-major
        wrap = stream.reshape(-1, 16).T.astype(np.int16)   # [16, CT*8]
        eidx16[c] = np.tile(wrap, (8, 1))

    # quad-packed x gather table (replicated) + per-core feature-major blocks
    xtab = np.zeros((TROWS, F), NPBF)
    xtab[trow] = x.astype(NPBF)
    xtabq = xtab.reshape(TROWS // 4, 4 * F)
    xfm = np.zeros((NCORES, F, PNP), NPBF)
    for c in range(NCORES):
        ids = np.arange(c * PN, (c + 1) * PN)
        xfm[c][:, rank[ids]] = x[ids].T.astype(NPBF)

    return dict(
        N=N, F=F, PN=PN, NB=NB, PNP=PNP, TROWS=TROWS, CT=CT,
        Lb=Lb.tolist(), cb_off=cb_off.tolist(), sl_blocks=sl_blocks,
        sl_rows=sl_rows, sl_base=sl_base, rank=rank,
        eidx16=eidx16, wk=wk, xtabq=xtabq, xfm=xfm,
    )


def _build(meta, HID, NCLS):
    F = meta["F"]
    NB, PNP, TROWS, CT = meta["NB"], meta["PNP"], meta["TROWS"], meta["CT"]
    Lb, cb_off = meta["Lb"], meta["cb_off"]
    sl_blocks = meta["sl_blocks"]
    sl_rows = [int(r) for r in meta["sl_rows"]]
    sl_base = [int(b) for b in meta["sl_base"]]
    QR = TROWS // 4
    E4 = 4 * F  # quad row elems (256)

    nc = bacc.Bacc(
        "TRN2", target_bir_lowering=False, debug=False,
        num_devices=NCORES, num_swdge_queues=NQ,
    )
    d_xtabq = nc.dram_tensor("xtabq", [QR, E4], BF16, kind="ExternalInput")
    d_xfm = nc.dram_tensor("xfm", [F, PNP], BF16, kind="ExternalInput")
    d_eidx = nc.dram_tensor("eidx", [P, CT * 8], I16, kind="ExternalInput")
    d_wk = nc.dram_tensor("wk", [P, 4 * CT], BF16, kind="ExternalInput")
    d_w10c = nc.dram_tensor("w10c", [F, HID], BF16, kind="ExternalInput")
    d_w11 = nc.dram_tensor("w11", [F, HID], BF16, kind="ExternalInput")
    d_w12x2 = nc.dram_tensor("w12x2", [F, HID], BF16, kind="ExternalInput")
    d_w2c0 = nc.dram_tensor("w2c0", [HID, NCLS], BF16, kind="ExternalInput")
    d_w21 = nc.dram_tensor("w21", [HID, NCLS], BF16, kind="ExternalInput")
    d_w22x2 = nc.dram_tensor("w22x2", [HID, NCLS], BF16, kind="ExternalInput")
    d_b1 = nc.dram_tensor("b1r", [1, HID], BF16, kind="ExternalInput")
    d_b2 = nc.dram_tensor("b2r", [1, NCLS], BF16, kind="ExternalInput")
    d_ones = nc.dram_tensor("ones1", [1, P], BF16, kind="ExternalInput")
    d_identf = nc.dram_tensor("identf", [P, P], F32, kind="ExternalInput")
    d_out = nc.dram_tensor("out", [PNP, NCLS], F32, kind="ExternalOutput")

    # u rows padded to F wide so the quad table stays 256-elem rows
    d_vloc = nc.dram_tensor("vloc", [PNP, HID], BF16)
    d_hloc = nc.dram_tensor("hloc", [PNP, HID], BF16)
    d_uloc = nc.dram_tensor("uloc", [PNP, F], BF16)
    d_t1fm = nc.dram_tensor("t1fm", [F, PNP], BF16)
    d_hfm = nc.dram_tensor("hfm", [HID, PNP], BF16)
    d_s1fm = nc.dram_tensor("s1fm", [HID, PNP], BF16)
    d_vfull = nc.dram_tensor("vfull", [QR, E4], BF16, addr_space="Shared")
    d_hfull = nc.dram_tensor("hfull", [QR, E4], BF16, addr_space="Shared")
    d_ufull = nc.dram_tensor("ufull", [QR, E4], BF16, addr_space="Shared")

    groups_cc = [list(range(NCORES))]
    qcounter = [0]

    # block groups for gather calls: consecutive blocks, sum(Lb) <= GMAX
    bgroups = []
    cur, acc = [], 0
    for b in range(NB):
        if cur and acc + Lb[b] > GMAX:
            bgroups.append(cur)
            cur, acc = [], 0
        cur.append(b)
        acc += Lb[b]
    if cur:
        bgroups.append(cur)

    sl_last = np.cumsum(sl_blocks) - 1

    with tile.TileContext(nc) as tc:
        with (
            tc.tile_pool(name="const", bufs=1) as constp,
            tc.tile_pool(name="xg", bufs=2) as xgp,
            tc.tile_pool(name="pr", bufs=2) as prp,
            tc.tile_pool(name="red", bufs=4) as redp,
            tc.tile_pool(name="fm", bufs=6) as fmp,
            tc.tile_pool(name="st", bufs=6) as stp,
            tc.tile_pool(name="sm", bufs=4) as smp,
            tc.tile_pool(name="ptr", bufs=2, space="PSUM") as ptrp,
            tc.tile_pool(name="pmm", bufs=2, space="PSUM") as pmmp,
        ):
            # ---- resident tiles ----
            eidx = constp.tile([P, CT * 8], I16, tag="eidx")
            wk = constp.tile([P, 4 * CT], BF16, tag="wk")
            w10c = constp.tile([F, HID], BF16, tag="w10c")
            w11 = constp.tile([F, HID], BF16, tag="w11")
            w12x2 = constp.tile([F, HID], BF16, tag="w12x2")
            w2c0 = constp.tile([HID, NCLS], BF16, tag="w2c0")
            w21 = constp.tile([HID, NCLS], BF16, tag="w21")
            w22x2 = constp.tile([HID, NCLS], BF16, tag="w22x2")
            b1r = constp.tile([1, HID], BF16, tag="b1r")
            b2r = constp.tile([1, NCLS], BF16, tag="b2r")
            ones1 = constp.tile([1, P], BF16, tag="ones1")
            identf = constp.tile([P, P], F32, tag="identf")

            nc.sync.dma_start(eidx[:], d_eidx[:])
            nc.sync.dma_start(wk[:], d_wk[:])
            nc.sync.dma_start(w10c[:], d_w10c[:])
            nc.sync.dma_start(w11[:], d_w11[:])
            nc.sync.dma_start(w12x2[:], d_w12x2[:])
            nc.sync.dma_start(w2c0[:], d_w2c0[:])
            nc.sync.dma_start(w21[:], d_w21[:])
            nc.sync.dma_start(w22x2[:], d_w22x2[:])
            nc.sync.dma_start(b1r[:], d_b1[:])
            nc.sync.dma_start(b2r[:], d_b2[:])
            nc.sync.dma_start(ones1[:], d_ones[:])
            nc.sync.dma_start(identf[:], d_identf[:])

            def ell_phase(ph, table, FW, per_block):
                """SpMM for one phase: per block group, gather quad rows with
                <=1024-idx dma_gather calls (SWDGE ring limit) + 4 lane
                multiplies, then one 4D reduce per block."""
                for gi, grp in enumerate(bgroups):
                    g0 = cb_off[grp[0]]
                    gc = cb_off[grp[-1] + 1] - g0
                    xg = xgp.tile([P, GMAX * E4], BF16, tag="xg",
                                  name=f"xg{ph}_{gi}")
                    xg_v = xg[:, : gc * E4].rearrange("p (c e) -> p c e", e=E4)
                    for j0 in range(0, gc, CPG):
                        jc = min(CPG, gc - j0)
                        nc.gpsimd.dma_gather(
                            out_ap=xg_v[:, j0 : j0 + jc, :],
                            in_ap=table[:],
                            idxs_ap=eidx[
                                :, (g0 + j0) * 8 : (g0 + j0 + jc) * 8
                            ],
                            num_idxs=jc * P, num_idxs_reg=jc * P,
                            elem_size=E4,
                            queue_num=qcounter[0] % NQ,
                        )
                        qcounter[0] += 1
                    pr = prp.tile([P, 4 * GMAX * F], BF16, tag="pr",
                                  name=f"pr{ph}_{gi}")
                    for k in range(4):
                        pk = pr[:, k * GMAX * F : k * GMAX * F + gc * F]
                        nc.vector.tensor_tensor(
                            out=pk.rearrange("p (c f) -> p c f", f=F),
                            in0=xg_v[:, :, k * F : k * F + F],
                            in1=wk[:, k * CT + g0 : k * CT + g0 + gc]
                                .to_broadcast([P, gc, F]),
                            op=ALU.mult,
                        )
                    prv = pr[:].rearrange("p (k c f) -> p f k c", k=4, f=F)
                    for b in grp:
                        c0 = cb_off[b] - g0
                        L = Lb[b]
                        red = redp.tile([P, F], F32, tag="red",
                                        name=f"red{ph}_{b}")
                        nc.vector.tensor_reduce(
                            red[:, :FW],
                            prv[:, :FW, :, c0 : c0 + L],
                            axis=AX.XY, op=ALU.add,
                        )
                        per_block(b, red[:, :FW])

            def fire_ag(b, loc, full):
                hits = np.flatnonzero(sl_last == b)
                if len(hits) == 0:
                    return
                s = int(hits[0])
                lr0 = sum(sl_rows[:s])
                lr1 = lr0 + sl_rows[s]
                q0 = sl_base[s] // 4
                q1 = q0 + sl_rows[s] * NCORES // 4
                nc.gpsimd.collective_compute(
                    "AllGather", ALU.bypass, replica_groups=groups_cc,
                    ins=[loc[lr0:lr1, :]], outs=[full[q0:q1, :]],
                )

            # ---- phase 1: T1 = A x;  v = T1 @ (2 W1[2]) ----
            def p1_block(b, red):
                tr = ptrp.tile([F, P], F32, tag="tr", name=f"p1tr{b}")
                nc.tensor.transpose(tr[:], red, identf[:])
                t1fm = fmp.tile([F, P], BF16, tag="fm", name=f"p1fm{b}")
                nc.scalar.activation(t1fm[:], tr[:], ACTF.Identity)
                nc.sync.dma_start(d_t1fm[:, b * P : (b + 1) * P], t1fm[:])
                vps = pmmp.tile([P, HID], F32, tag="mm", name=f"p1v{b}")
                nc.tensor.matmul(vps[:], lhsT=t1fm[:], rhs=w12x2[:],
                                 start=True, stop=True)
                vbf = stp.tile([P, HID], BF16, tag="st64", name=f"p1vb{b}")
                nc.scalar.activation(vbf[:], vps[:], ACTF.Identity)
                nc.sync.dma_start(d_vloc[b * P : (b + 1) * P, :], vbf[:])
                fire_ag(b, d_vloc, d_vfull)

            ell_phase(1, d_xtabq, F, p1_block)

            # ---- phase 2: t2p = A v;  h = relu(x W10c + T1 W11 + b1 + t2p)
            def p2_block(b, red):
                xfmb = fmp.tile([F, P], BF16, tag="fm", name=f"p2x{b}")
                nc.sync.dma_start(xfmb[:], d_xfm[:, b * P : (b + 1) * P])
                t1fmb = fmp.tile([F, P], BF16, tag="fm", name=f"p2t{b}")
                nc.sync.dma_start(t1fmb[:], d_t1fm[:, b * P : (b + 1) * P])
                o1 = pmmp.tile([P, HID], F32, tag="mm", name=f"p2o{b}")
                nc.tensor.matmul(o1[:], lhsT=xfmb[:], rhs=w10c[:],
                                 start=True, stop=False)
                nc.tensor.matmul(o1[:], lhsT=t1fmb[:], rhs=w11[:],
                                 start=False, stop=False)
                nc.tensor.matmul(o1[:], lhsT=ones1[:], rhs=b1r[:],
                                 start=False, stop=True)
                hpre = stp.tile([P, HID], F32, tag="hpre", name=f"p2hp{b}")
                nc.vector.tensor_tensor(out=hpre[:], in0=o1[:], in1=red,
                                        op=ALU.add)
                hrm = stp.tile([P, HID], BF16, tag="st64", name=f"p2hr{b}")
                nc.scalar.activation(hrm[:], hpre[:], ACTF.Relu)
                nc.sync.dma_start(d_hloc[b * P : (b + 1) * P, :], hrm[:])
                tr = ptrp.tile([HID, P], F32, tag="tr", name=f"p2tr{b}")
                nc.tensor.transpose(tr[:], hpre[:], identf[:])
                hfm = fmp.tile([HID, P], BF16, tag="fm", name=f"p2hf{b}")
                nc.scalar.activation(hfm[:], tr[:], ACTF.Relu)
                nc.sync.dma_start(d_hfm[:, b * P : (b + 1) * P], hfm[:])
                fire_ag(b, d_hloc, d_hfull)

            ell_phase(2, d_vfull, HID, p2_block)

            # ---- phase 3: S1 = A h;  u = S1 @ (2 W2[2]) (padded to F) ----
            def p3_block(b, red):
                tr = ptrp.tile([HID, P], F32, tag="tr", name=f"p3tr{b}")
                nc.tensor.transpose(tr[:], red, identf[:])
                s1fm = fmp.tile([HID, P], BF16, tag="fm", name=f"p3fm{b}")
                nc.scalar.activation(s1fm[:], tr[:], ACTF.Identity)
                nc.sync.dma_start(d_s1fm[:, b * P : (b + 1) * P], s1fm[:])
                ups = pmmp.tile([P, NCLS], F32, tag="mm", name=f"p3u{b}")
                nc.tensor.matmul(ups[:], lhsT=s1fm[:], rhs=w22x2[:],
                                 start=True, stop=True)
                ubf = stp.tile([P, F], BF16, tag="stu", name=f"p3ub{b}")
                nc.gpsimd.memset(ubf[:, NCLS:], 0.0)
                nc.scalar.activation(ubf[:, :NCLS], ups[:], ACTF.Identity)
                nc.sync.dma_start(d_uloc[b * P : (b + 1) * P, :], ubf[:])
                fire_ag(b, d_uloc, d_ufull)

            ell_phase(3, d_hfull, HID, p3_block)

            # ---- phase 4: s2p = A u; out = softmax(h W2c0 + S1 W21 + b2
            #                                         + s2p) ----
            def p4_block(b, red):
                hfmb = fmp.tile([HID, P], BF16, tag="fm", name=f"p4h{b}")
                nc.sync.dma_start(hfmb[:], d_hfm[:, b * P : (b + 1) * P])
                s1fmb = fmp.tile([HID, P], BF16, tag="fm", name=f"p4s{b}")
                nc.sync.dma_start(s1fmb[:], d_s1fm[:, b * P : (b + 1) * P])
                o2 = pmmp.tile([P, NCLS], F32, tag="mm", name=f"p4o{b}")
                nc.tensor.matmul(o2[:], lhsT=hfmb[:], rhs=w2c0[:],
                                 start=True, stop=False)
                nc.tensor.matmul(o2[:], lhsT=s1fmb[:], rhs=w21[:],
                                 start=False, stop=False)
                nc.tensor.matmul(o2[:], lhsT=ones1[:], rhs=b2r[:],
                                 start=False, stop=True)
                opre = smp.tile([P, NCLS], F32, tag="opre", name=f"p4op{b}")
                nc.vector.tensor_tensor(out=opre[:], in0=o2[:], in1=red,
                                        op=ALU.add)
                negm = smp.tile([P, 1], F32, tag="negm", name=f"p4nm{b}")
                nc.vector.tensor_reduce(
                    negm[:], opre[:], axis=AX.X, op=ALU.max, negate=True
                )
                ex = smp.tile([P, NCLS], F32, tag="ex", name=f"p4ex{b}")
                nc.scalar.activation(ex[:], opre[:], ACTF.Exp, bias=negm[:])
                ssum = smp.tile([P, 1], F32, tag="ssum", name=f"p4ss{b}")
                nc.vector.tensor_reduce(ssum[:], ex[:], axis=AX.X, op=ALU.add)
                rcp = smp.tile([P, 1], F32, tag="rcp", name=f"p4rc{b}")
                nc.vector.reciprocal(rcp[:], ssum[:])
                res = smp.tile([P, NCLS], F32, tag="res", name=f"p4rs{b}")
                nc.vector.tensor_scalar(
                    out=res[:], in0=ex[:], scalar1=rcp[:, :1], scalar2=None,
                    op0=ALU.mult,
                )
                nc.sync.dma_start(d_out[b * P : (b + 1) * P, :], res[:])

            ell_phase(4, d_ufull, NCLS, p4_block)

    nc.compile()
    return nc


def kernel(x, edge_index, W1, b1, W2, b2, _backend="hw"):
    x = np.asarray(x, dtype=np.float32)
    edge_index = np.asarray(edge_index, dtype=np.int32)
    W1 = np.asarray(W1, dtype=np.float32)
    b1 = np.asarray(b1, dtype=np.float32)
    W2 = np.asarray(W2, dtype=np.float32)
    b2 = np.asarray(b2, dtype=np.float32)
    K, F, HID = W1.shape
    NCLS = W2.shape[2]

    meta = _preprocess(x, edge_index)
    nc = _build(meta, HID, NCLS)

    CT = meta["CT"]
    in_common = {
        "xtabq": meta["xtabq"],
        "w10c": (W1[0] - W1[2]).astype(NPBF),
        "w11": W1[1].astype(NPBF),
        "w12x2": (2.0 * W1[2]).astype(NPBF),
        "w2c0": (W2[0] - W2[2]).astype(NPBF),
        "w21": W2[1].astype(NPBF),
        "w22x2": (2.0 * W2[2]).astype(NPBF),
        "b1r": b1.reshape(1, -1).astype(NPBF),
        "b2r": b2.reshape(1, -1).astype(NPBF),
        "ones1": np.ones((1, P), NPBF),
        "identf": np.eye(P, dtype=np.float32),
    }
    in_maps = []
    for c in range(NCORES):
        m = dict(in_common)
        m["xfm"] = meta["xfm"][c]
        m["eidx"] = meta["eidx16"][c]
        m["wk"] = (
            meta["wk"][c].transpose(1, 0, 2).reshape(P, 4 * CT).astype(NPBF)
        )
        in_maps.append(m)

    N = meta["N"]
    PN = meta["PN"]
    rank = meta["rank"]
    out = np.empty((N, NCLS), np.float32)

    if _backend == "sim":
        from concourse.bass_interp import MultiCoreSim

        sim = MultiCoreSim(nc, num_cores=NCORES)
        for c in range(NCORES):
            for name, arr in in_maps[c].items():
                sim.cores[c].tensor(name)[:] = arr
        sim.simulate()
        for c in range(NCORES):
            ids = np.arange(c * PN, (c + 1) * PN)
            res = np.array(sim.cores[c].tensor("out"))
            out[ids] = res[rank[ids]]
        kernel.last_result = None
        return out

    trace = bool(os.environ.get("BASS_TRACE"))
    res = run_bass_kernel_spmd(
        nc, in_maps, core_ids=list(range(NCORES)), trace=trace
    )
    kernel.last_result = res
    for c in range(NCORES):
        ids = np.arange(c * PN, (c + 1) * PN)
        out[ids] = res.results[c]["out"][rank[ids]]
    return out


# revision 13
# speedup vs baseline: 1.8439x; 1.2104x over previous
"""ChebConv (K=3, two layers + softmax) GNN kernel for 8 Trainium2 NeuronCores.

Strategy (dst-node sharding, ELL-format SpMM, quad-packed bf16 tables):
  - Nodes are split into 8 contiguous shards (one per core); each core owns the
    edges whose *destination* lies in its shard.
  - Within each core, nodes are PERMUTED by descending in-degree so that each
    128-node dst block has near-uniform degree; edges are laid out in ELL
    format: slot grid [128 dst-partitions x L_b chunks] per block, padded with
    (idx=0, w=0).  Chunk counts L_b are maxed across cores so one SPMD program
    serves all 8.
  - Gather tables are QUAD-PACKED bf16: [TROWS/4, 256] -- 4 nodes per 512B row
    so the row index fits dma_gather's int16 (25088 < 32768).  One 512B
    descriptor per edge slot; the needed sub-node is selected by 4 masked
    per-edge weight arrays wk[k] (w where trow%4==k else 0).
  - SpMM per ~GMAX-chunk group: one dma_gather + 4 DVE broadcast-multiplies
    (one per sub-node lane) into a [128, 4, GMAX, 64] product buffer, then ONE
    strided 4D DVE reduce per block over (lane, slot) -> row-major [128, F].
    No per-chunk one-hot builds, no PE matmuls in the SpMM.
  - Chebyshev algebra is refactored so every phase gathers a fresh table with
    the SAME edge weights (the 2x of T2 = 2 A T1 - T0 is folded into W):
      h   = relu(x(W1_0-W1_2) + T1 W1_1 + A v + b1),   v = T1 (2 W1_2)
      out = softmax(h(W2_0-W2_2) + S1 W2_1 + A u + b2), u = S1 (2 W2_2)
    Phase tables: x, v, h (64-wide), u (40-wide, padded to 64 in the quad).
  - Dense per-block work: PE transpose + 3 small bf16 matmuls (biases folded
    in via a ones-row matmul).  Feature-major copies bounce through DRAM.
  - v/h/u tables are exchanged with AllGather collectives split into 2 slices
    each so the first slice overlaps the tail of the producing phase.
"""

import os

import numpy as np
import ml_dtypes

import concourse.bass as bass
import concourse.mybir as mybir
import concourse.tile as tile
from concourse import bacc
from concourse.bass_utils import run_bass_kernel_spmd

NCORES = 8
P = 128
GMAX = 48  # ELL chunks per gather group (one product buffer)
CPG = 8    # chunks per dma_gather call (1024 idxs -- SWDGE ring limit)
NQ = 4     # SWDGE queues round-robin

BF16 = mybir.dt.bfloat16
F32 = mybir.dt.float32
I16 = mybir.dt.int16
ALU = mybir.AluOpType
ACTF = mybir.ActivationFunctionType
AX = mybir.AxisListType
NPBF = ml_dtypes.bfloat16


def _preprocess(x, edge_index):
    N, F = x.shape
    assert N % NCORES == 0
    PN = N // NCORES
    NB = (PN + P - 1) // P
    PNP = NB * P
    TROWS = NCORES * PNP
    assert TROWS % 4 == 0 and TROWS // 4 < 32768

    src = edge_index[0].astype(np.int64)
    dst = edge_index[1].astype(np.int64)
    keep = src != dst
    deg = np.bincount(src[keep], minlength=N).astype(np.float64)
    dis = np.where(deg > 0, 1.0 / np.sqrt(np.maximum(deg, 1.0)), 0.0)
    w = np.where(keep, -dis[src] * dis[dst], 0.0).astype(np.float32)
    live = w != 0.0
    src, dst, w = src[live], dst[live], w[live]

    # per-core rank of each node: descending in-degree within its core
    indeg = np.bincount(dst, minlength=N).astype(np.int64)
    rank = np.empty(N, np.int64)
    for c in range(NCORES):
        ids = np.arange(c * PN, (c + 1) * PN)
        order = np.argsort(-indeg[ids], kind="stable")
        rank[ids[order]] = np.arange(PN)

    # AllGather slice structure (2 slices of whole blocks; rows %4 == 0)
    if NB >= 2:
        sl_blocks = [NB // 2, NB - NB // 2]
    else:
        sl_blocks = [NB]
    sl_rows = np.array([b * P for b in sl_blocks], np.int64)
    sl_local0 = np.concatenate([[0], np.cumsum(sl_rows)])[:-1]
    sl_base = np.concatenate([[0], np.cumsum(sl_rows * NCORES)])[:-1]

    c_all = np.arange(N) // PN
    r_all = rank
    s_all = np.searchsorted(np.cumsum(sl_rows), r_all, side="right")
    trow = sl_base[s_all] + c_all * sl_rows[s_all] + (r_all - sl_local0[s_all])

    # ELL slot grid
    blk = (rank[dst] // P).astype(np.int64)
    part = (rank[dst] % P).astype(np.int64)
    order = np.argsort(dst, kind="stable")
    sdst = dst[order]
    starts = np.flatnonzero(np.r_[True, sdst[1:] != sdst[:-1]])
    seg_len = np.diff(np.r_[starts, len(sdst)])
    ccount = np.arange(len(sdst)) - np.repeat(starts, seg_len)
    slot = np.empty(len(dst), np.int64)
    slot[order] = ccount

    indeg_by_rank = np.zeros(NCORES * PNP, np.int64)
    indeg_by_rank[c_all * PNP + r_all] = indeg
    Lb = indeg_by_rank.reshape(NCORES, NB, P).max(axis=(0, 2))
    Lb = np.maximum(Lb, 1)

    # gather groups of consecutive blocks with a UNIFORM chunk count LG
    # (= L of the group's first block, since Lb is non-increasing) so the
    # chunk-tree reduce can run on a regular [128, NBG, LG, F] view.
    groups = []  # (col0, LG, [blocks])
    col0 = 0
    b = 0
    while b < NB:
        LG = int(Lb[b])
        nbg = 1
        while b + nbg < NB and (nbg + 1) * LG <= GMAX:
            nbg += 1
        groups.append((col0, LG, list(range(b, b + nbg))))
        col0 += nbg * LG
        b += nbg
    CT = col0
    blk_col0 = np.zeros(NB, np.int64)  # first chunk column of each block
    blk_LG = np.zeros(NB, np.int64)
    for g0, LG, blks in groups:
        for i, bb in enumerate(blks):
            blk_col0[bb] = g0 + i * LG
            blk_LG[bb] = LG

    core_of = dst // PN
    col = blk_col0[blk] + slot
    qidx = np.zeros((NCORES, P, CT), np.int64)      # quad row of src
    wk = np.zeros((NCORES, 4, P, CT), np.float32)   # lane-masked weights
    qidx[core_of, part, col] = trow[src] // 4
    wk[core_of, trow[src] % 4, part, col] = w

    # wrapped int16 index stream per core: position i=(c*128+d) -> [i%16,i//16]
    eidx16 = np.zeros((NCORES, P, CT * 8), np.int16)
    for c in range(NCORES):
        stream = qidx[c].T.reshape(-1)              # chunk-major
        wrap = stream.reshape(-1, 16).T.astype(np.int16)   # [16, CT*8]
        eidx16[c] = np.tile(wrap, (8, 1))

    # quad-packed x gather table (replicated) + per-core feature-major blocks
    xtab = np.zeros((TROWS, F), NPBF)
    xtab[trow] = x.astype(NPBF)
    xtabq = xtab.reshape(TROWS // 4, 4 * F)
    xfm = np.zeros((NCORES, F, PNP), NPBF)
    for c in range(NCORES):
        ids = np.arange(c * PN, (c + 1) * PN)
        xfm[c][:, rank[ids]] = x[ids].T.astype(NPBF)

    GM = max(LG * len(blks) for _, LG, blks in groups)
    return dict(
        N=N, F=F, PN=PN, NB=NB, PNP=PNP, TROWS=TROWS, CT=CT, GM=GM,
        groups=groups, sl_blocks=sl_blocks,
        sl_rows=sl_rows, sl_base=sl_base, rank=rank,
        eidx16=eidx16, wk=wk, xtabq=xtabq, xfm=xfm,
    )


def _build(meta, HID, NCLS):
    F = meta["F"]
    NB, PNP, TROWS, CT = meta["NB"], meta["PNP"], meta["TROWS"], meta["CT"]
    GM = meta["GM"]
    groups = meta["groups"]
    sl_blocks = meta["sl_blocks"]
    sl_rows = [int(r) for r in meta["sl_rows"]]
    sl_base = [int(b) for b in meta["sl_base"]]
    QR = TROWS // 4
    E4 = 4 * F  # quad row elems (256)

    nc = bacc.Bacc(
        "TRN2", target_bir_lowering=False, debug=False,
        num_devices=NCORES, num_swdge_queues=NQ,
    )
    d_xtabq = nc.dram_tensor("xtabq", [QR, E4], BF16, kind="ExternalInput")
    d_xfm = nc.dram_tensor("xfm", [F, PNP], BF16, kind="ExternalInput")
    d_eidx = nc.dram_tensor("eidx", [P, CT * 8], I16, kind="ExternalInput")
    d_wk = nc.dram_tensor("wk", [P, 4 * CT], BF16, kind="ExternalInput")
    d_w10c = nc.dram_tensor("w10c", [F, HID], BF16, kind="ExternalInput")
    d_w11 = nc.dram_tensor("w11", [F, HID], BF16, kind="ExternalInput")
    d_w12x2 = nc.dram_tensor("w12x2", [F, HID], BF16, kind="ExternalInput")
    d_w2c0 = nc.dram_tensor("w2c0", [HID, NCLS], BF16, kind="ExternalInput")
    d_w21 = nc.dram_tensor("w21", [HID, NCLS], BF16, kind="ExternalInput")
    d_w22x2 = nc.dram_tensor("w22x2", [HID, NCLS], BF16, kind="ExternalInput")
    d_b1 = nc.dram_tensor("b1r", [1, HID], BF16, kind="ExternalInput")
    d_b2 = nc.dram_tensor("b2r", [1, NCLS], BF16, kind="ExternalInput")
    d_ones = nc.dram_tensor("ones1", [1, P], BF16, kind="ExternalInput")
    d_identf = nc.dram_tensor("identf", [P, P], F32, kind="ExternalInput")
    d_out = nc.dram_tensor("out", [PNP, NCLS], F32, kind="ExternalOutput")

    # u rows padded to F wide so the quad table stays 256-elem rows
    d_vloc = nc.dram_tensor("vloc", [PNP, HID], BF16)
    d_hloc = nc.dram_tensor("hloc", [PNP, HID], BF16)
    d_uloc = nc.dram_tensor("uloc", [PNP, F], BF16)
    d_t1fm = nc.dram_tensor("t1fm", [F, PNP], BF16)
    d_hfm = nc.dram_tensor("hfm", [HID, PNP], BF16)
    d_s1fm = nc.dram_tensor("s1fm", [HID, PNP], BF16)
    d_vfull = nc.dram_tensor("vfull", [QR, E4], BF16, addr_space="Shared")
    d_hfull = nc.dram_tensor("hfull", [QR, E4], BF16, addr_space="Shared")
    d_ufull = nc.dram_tensor("ufull", [QR, E4], BF16, addr_space="Shared")

    groups_cc = [list(range(NCORES))]
    qcounter = [0]

    sl_last = np.cumsum(sl_blocks) - 1

    with tile.TileContext(nc) as tc:
        with (
            tc.tile_pool(name="const", bufs=1) as constp,
            tc.tile_pool(name="xg", bufs=3) as xgp,
            tc.tile_pool(name="f1", bufs=2) as f1p,
            tc.tile_pool(name="ls", bufs=2) as lsp,
            tc.tile_pool(name="red", bufs=4) as redp,
            tc.tile_pool(name="fm", bufs=6) as fmp,
            tc.tile_pool(name="st", bufs=6) as stp,
            tc.tile_pool(name="sm", bufs=4) as smp,
            tc.tile_pool(name="ptr", bufs=2, space="PSUM") as ptrp,
            tc.tile_pool(name="pmm", bufs=2, space="PSUM") as pmmp,
        ):
            # ---- resident tiles ----
            eidx = constp.tile([P, CT * 8], I16, tag="eidx")
            wk = constp.tile([P, 4 * CT], BF16, tag="wk")
            w10c = constp.tile([F, HID], BF16, tag="w10c")
            w11 = constp.tile([F, HID], BF16, tag="w11")
            w12x2 = constp.tile([F, HID], BF16, tag="w12x2")
            w2c0 = constp.tile([HID, NCLS], BF16, tag="w2c0")
            w21 = constp.tile([HID, NCLS], BF16, tag="w21")
            w22x2 = constp.tile([HID, NCLS], BF16, tag="w22x2")
            b1r = constp.tile([1, HID], BF16, tag="b1r")
            b2r = constp.tile([1, NCLS], BF16, tag="b2r")
            ones1 = constp.tile([1, P], BF16, tag="ones1")
            identf = constp.tile([P, P], F32, tag="identf")

            nc.sync.dma_start(eidx[:], d_eidx[:])
            nc.sync.dma_start(wk[:], d_wk[:])
            nc.sync.dma_start(w10c[:], d_w10c[:])
            nc.sync.dma_start(w11[:], d_w11[:])
            nc.sync.dma_start(w12x2[:], d_w12x2[:])
            nc.sync.dma_start(w2c0[:], d_w2c0[:])
            nc.sync.dma_start(w21[:], d_w21[:])
            nc.sync.dma_start(w22x2[:], d_w22x2[:])
            nc.sync.dma_start(b1r[:], d_b1[:])
            nc.sync.dma_start(b2r[:], d_b2[:])
            nc.sync.dma_start(ones1[:], d_ones[:])
            nc.sync.dma_start(identf[:], d_identf[:])

            def ell_phase(ph, table, FW, per_block):
                """SpMM for one phase.  Per gather group: <=1024-idx
                dma_gather calls of quad rows; one in-place weighted multiply
                (lane-masked weights via a stride-0 broadcast view); two
                dense lane-fold adds 256->128->64; then a contiguous
                tree-add over the group's uniform LG chunks."""
                for gi, (g0, LG, blks) in enumerate(groups):
                    nbg = len(blks)
                    gc = nbg * LG
                    xg = xgp.tile([P, GM * E4], BF16, tag="xg",
                                  name=f"xg{ph}_{gi}")
                    xg_v = xg[:, : gc * E4].rearrange("p (c e) -> p c e", e=E4)
                    for j0 in range(0, gc, CPG):
                        jc = min(CPG, gc - j0)
                        nc.gpsimd.dma_gather(
                            out_ap=xg_v[:, j0 : j0 + jc, :],
                            in_ap=table[:],
                            idxs_ap=eidx[
                                :, (g0 + j0) * 8 : (g0 + j0 + jc) * 8
                            ],
                            num_idxs=jc * P, num_idxs_reg=jc * P,
                            elem_size=E4,
                            queue_num=qcounter[0] % NQ,
                        )
                        qcounter[0] += 1
                    # weighted multiply, in place over the whole quad group
                    xg_q = xg[:, : gc * E4].rearrange(
                        "p (c k f) -> p c k f", k=4, f=F
                    )
                    wk_v = (
                        wk[:].rearrange("p (k c) -> p c k", k=4)
                        [:, g0 : g0 + gc, :].to_broadcast([P, gc, 4, F])
                    )
                    nc.vector.tensor_tensor(
                        out=xg_q, in0=xg_q, in1=wk_v, op=ALU.mult
                    )
                    # lane folds: 256 -> 128 -> 64 (dense outputs)
                    f1 = f1p.tile([P, GM * 2 * F], BF16, tag="f1",
                                  name=f"f1_{ph}_{gi}")
                    f1_v = f1[:, : gc * 2 * F].rearrange(
                        "p (c k f) -> p c k f", k=2, f=F
                    )
                    nc.vector.tensor_tensor(
                        out=f1_v, in0=xg_q[:, :, 0:2, :],
                        in1=xg_q[:, :, 2:4, :], op=ALU.add,
                    )
                    ls = lsp.tile([P, GM * F], BF16, tag="ls",
                                  name=f"ls_{ph}_{gi}")
                    ls_v = ls[:, : gc * F].rearrange("p (c f) -> p c f", f=F)
                    nc.vector.tensor_tensor(
                        out=ls_v, in0=f1_v[:, :, 0, :], in1=f1_v[:, :, 1, :],
                        op=ALU.add,
                    )
                    # tree-add over the LG chunks of every block at once
                    lsb = ls[:, : gc * F].rearrange(
                        "p (b c f) -> p b c f", b=nbg, f=F
                    )
                    l = LG
                    while l > 1:
                        a = l // 2
                        nc.vector.tensor_tensor(
                            out=lsb[:, :, :a, :], in0=lsb[:, :, :a, :],
                            in1=lsb[:, :, a : 2 * a, :], op=ALU.add,
                        )
                        if l % 2:
                            nc.vector.tensor_tensor(
                                out=lsb[:, :, 0:1, :], in0=lsb[:, :, 0:1, :],
                                in1=lsb[:, :, l - 1 : l, :], op=ALU.add,
                            )
                        l = a
                    red = redp.tile([P, nbg * F], F32, tag="red",
                                    name=f"red{ph}_{gi}")
                    nc.vector.tensor_copy(
                        red[:].rearrange("p (b f) -> p b f", f=F),
                        lsb[:, :, 0, :],
                    )
                    for i, b in enumerate(blks):
                        per_block(b, red[:, i * F : i * F + FW])

            def fire_ag(b, loc, full):
                hits = np.flatnonzero(sl_last == b)
                if len(hits) == 0:
                    return
                s = int(hits[0])
                lr0 = sum(sl_rows[:s])
                lr1 = lr0 + sl_rows[s]
                q0 = sl_base[s] // 4
                q1 = q0 + sl_rows[s] * NCORES // 4
                nc.gpsimd.collective_compute(
                    "AllGather", ALU.bypass, replica_groups=groups_cc,
                    ins=[loc[lr0:lr1, :]], outs=[full[q0:q1, :]],
                )

            # ---- phase 1: T1 = A x;  v = T1 @ (2 W1[2]) ----
            def p1_block(b, red):
                tr = ptrp.tile([F, P], F32, tag="tr", name=f"p1tr{b}")
                nc.tensor.transpose(tr[:], red, identf[:])
                t1fm = fmp.tile([F, P], BF16, tag="fm", name=f"p1fm{b}")
                nc.scalar.activation(t1fm[:], tr[:], ACTF.Identity)
                nc.sync.dma_start(d_t1fm[:, b * P : (b + 1) * P], t1fm[:])
                vps = pmmp.tile([P, HID], F32, tag="mm", name=f"p1v{b}")
                nc.tensor.matmul(vps[:], lhsT=t1fm[:], rhs=w12x2[:],
                                 start=True, stop=True)
                vbf = stp.tile([P, HID], BF16, tag="st64", name=f"p1vb{b}")
                nc.scalar.activation(vbf[:], vps[:], ACTF.Identity)
                nc.sync.dma_start(d_vloc[b * P : (b + 1) * P, :], vbf[:])
                fire_ag(b, d_vloc, d_vfull)

            ell_phase(1, d_xtabq, F, p1_block)

            # ---- phase 2: t2p = A v;  h = relu(x W10c + T1 W11 + b1 + t2p)
            def p2_block(b, red):
                xfmb = fmp.tile([F, P], BF16, tag="fm", name=f"p2x{b}")
                nc.sync.dma_start(xfmb[:], d_xfm[:, b * P : (b + 1) * P])
                t1fmb = fmp.tile([F, P], BF16, tag="fm", name=f"p2t{b}")
                nc.sync.dma_start(t1fmb[:], d_t1fm[:, b * P : (b + 1) * P])
                o1 = pmmp.tile([P, HID], F32, tag="mm", name=f"p2o{b}")
                nc.tensor.matmul(o1[:], lhsT=xfmb[:], rhs=w10c[:],
                                 start=True, stop=False)
                nc.tensor.matmul(o1[:], lhsT=t1fmb[:], rhs=w11[:],
                                 start=False, stop=False)
                nc.tensor.matmul(o1[:], lhsT=ones1[:], rhs=b1r[:],
                                 start=False, stop=True)
                hpre = stp.tile([P, HID], F32, tag="hpre", name=f"p2hp{b}")
                nc.vector.tensor_tensor(out=hpre[:], in0=o1[:], in1=red,
                                        op=ALU.add)
                hrm = stp.tile([P, HID], BF16, tag="st64", name=f"p2hr{b}")
                nc.scalar.activation(hrm[:], hpre[:], ACTF.Relu)
                nc.sync.dma_start(d_hloc[b * P : (b + 1) * P, :], hrm[:])
                tr = ptrp.tile([HID, P], F32, tag="tr", name=f"p2tr{b}")
                nc.tensor.transpose(tr[:], hpre[:], identf[:])
                hfm = fmp.tile([HID, P], BF16, tag="fm", name=f"p2hf{b}")
                nc.scalar.activation(hfm[:], tr[:], ACTF.Relu)
                nc.sync.dma_start(d_hfm[:, b * P : (b + 1) * P], hfm[:])
                fire_ag(b, d_hloc, d_hfull)

            ell_phase(2, d_vfull, HID, p2_block)

            # ---- phase 3: S1 = A h;  u = S1 @ (2 W2[2]) (padded to F) ----
            def p3_block(b, red):
                tr = ptrp.tile([HID, P], F32, tag="tr", name=f"p3tr{b}")
                nc.tensor.transpose(tr[:], red, identf[:])
                s1fm = fmp.tile([HID, P], BF16, tag="fm", name=f"p3fm{b}")
                nc.scalar.activation(s1fm[:], tr[:], ACTF.Identity)
                nc.sync.dma_start(d_s1fm[:, b * P : (b + 1) * P], s1fm[:])
                ups = pmmp.tile([P, NCLS], F32, tag="mm", name=f"p3u{b}")
                nc.tensor.matmul(ups[:], lhsT=s1fm[:], rhs=w22x2[:],
                                 start=True, stop=True)
                ubf = stp.tile([P, F], BF16, tag="stu", name=f"p3ub{b}")
                nc.gpsimd.memset(ubf[:, NCLS:], 0.0)
                nc.scalar.activation(ubf[:, :NCLS], ups[:], ACTF.Identity)
                nc.sync.dma_start(d_uloc[b * P : (b + 1) * P, :], ubf[:])
                fire_ag(b, d_uloc, d_ufull)

            ell_phase(3, d_hfull, HID, p3_block)

            # ---- phase 4: s2p = A u; out = softmax(h W2c0 + S1 W21 + b2
            #                                         + s2p) ----
            def p4_block(b, red):
                hfmb = fmp.tile([HID, P], BF16, tag="fm", name=f"p4h{b}")
                nc.sync.dma_start(hfmb[:], d_hfm[:, b * P : (b + 1) * P])
                s1fmb = fmp.tile([HID, P], BF16, tag="fm", name=f"p4s{b}")
                nc.sync.dma_start(s1fmb[:], d_s1fm[:, b * P : (b + 1) * P])
                o2 = pmmp.tile([P, NCLS], F32, tag="mm", name=f"p4o{b}")
                nc.tensor.matmul(o2[:], lhsT=hfmb[:], rhs=w2c0[:],
                                 start=True, stop=False)
                nc.tensor.matmul(o2[:], lhsT=s1fmb[:], rhs=w21[:],
                                 start=False, stop=False)
                nc.tensor.matmul(o2[:], lhsT=ones1[:], rhs=b2r[:],
                                 start=False, stop=True)
                opre = smp.tile([P, NCLS], F32, tag="opre", name=f"p4op{b}")
                nc.vector.tensor_tensor(out=opre[:], in0=o2[:], in1=red,
                                        op=ALU.add)
                negm = smp.tile([P, 1], F32, tag="negm", name=f"p4nm{b}")
                nc.vector.tensor_reduce(
                    negm[:], opre[:], axis=AX.X, op=ALU.max, negate=True
                )
                ex = smp.tile([P, NCLS], F32, tag="ex", name=f"p4ex{b}")
                nc.scalar.activation(ex[:], opre[:], ACTF.Exp, bias=negm[:])
                ssum = smp.tile([P, 1], F32, tag="ssum", name=f"p4ss{b}")
                nc.vector.tensor_reduce(ssum[:], ex[:], axis=AX.X, op=ALU.add)
                rcp = smp.tile([P, 1], F32, tag="rcp", name=f"p4rc{b}")
                nc.vector.reciprocal(rcp[:], ssum[:])
                res = smp.tile([P, NCLS], F32, tag="res", name=f"p4rs{b}")
                nc.vector.tensor_scalar(
                    out=res[:], in0=ex[:], scalar1=rcp[:, :1], scalar2=None,
                    op0=ALU.mult,
                )
                nc.sync.dma_start(d_out[b * P : (b + 1) * P, :], res[:])

            ell_phase(4, d_ufull, NCLS, p4_block)

    nc.compile()
    return nc


def kernel(x, edge_index, W1, b1, W2, b2, _backend="hw"):
    x = np.asarray(x, dtype=np.float32)
    edge_index = np.asarray(edge_index, dtype=np.int32)
    W1 = np.asarray(W1, dtype=np.float32)
    b1 = np.asarray(b1, dtype=np.float32)
    W2 = np.asarray(W2, dtype=np.float32)
    b2 = np.asarray(b2, dtype=np.float32)
    K, F, HID = W1.shape
    NCLS = W2.shape[2]

    meta = _preprocess(x, edge_index)
    nc = _build(meta, HID, NCLS)

    CT = meta["CT"]
    in_common = {
        "xtabq": meta["xtabq"],
        "w10c": (W1[0] - W1[2]).astype(NPBF),
        "w11": W1[1].astype(NPBF),
        "w12x2": (2.0 * W1[2]).astype(NPBF),
        "w2c0": (W2[0] - W2[2]).astype(NPBF),
        "w21": W2[1].astype(NPBF),
        "w22x2": (2.0 * W2[2]).astype(NPBF),
        "b1r": b1.reshape(1, -1).astype(NPBF),
        "b2r": b2.reshape(1, -1).astype(NPBF),
        "ones1": np.ones((1, P), NPBF),
        "identf": np.eye(P, dtype=np.float32),
    }
    in_maps = []
    for c in range(NCORES):
        m = dict(in_common)
        m["xfm"] = meta["xfm"][c]
        m["eidx"] = meta["eidx16"][c]
        m["wk"] = (
            meta["wk"][c].transpose(1, 0, 2).reshape(P, 4 * CT).astype(NPBF)
        )
        in_maps.append(m)

    N = meta["N"]
    PN = meta["PN"]
    rank = meta["rank"]
    out = np.empty((N, NCLS), np.float32)

    if _backend == "sim":
        from concourse.bass_interp import MultiCoreSim

        sim = MultiCoreSim(nc, num_cores=NCORES)
        for c in range(NCORES):
            for name, arr in in_maps[c].items():
                sim.cores[c].tensor(name)[:] = arr
        sim.simulate()
        for c in range(NCORES):
            ids = np.arange(c * PN, (c + 1) * PN)
            res = np.array(sim.cores[c].tensor("out"))
            out[ids] = res[rank[ids]]
        kernel.last_result = None
        return out

    trace = bool(os.environ.get("BASS_TRACE"))
    res = run_bass_kernel_spmd(
        nc, in_maps, core_ids=list(range(NCORES)), trace=trace
    )
    kernel.last_result = res
    for c in range(NCORES):
        ids = np.arange(c * PN, (c + 1) * PN)
        out[ids] = res.results[c]["out"][rank[ids]]
    return out
